# revision 3
# baseline (speedup 1.0000x reference)
"""Longformer attention TP-sharded Bass kernel for 8 NeuronCores.

Sharding: tensor-parallel over heads. Core d owns heads 2d, 2d+1:
  - Wq/Wk/Wv rows [128d:128(d+1)]  (nn.Linear: q = x @ Wq.T)
  - Wo columns [128d:128(d+1)]
  Each core computes its heads' sparse (windowed+global) attention and a
  full-size out-proj partial; host sums the 8 partials (the "all-reduce").

Device layout (all bf16 compute, fp32 PSUM accumulate):
  xT  [1024h, 4096s]  - x transposed (host prep) so hidden is contraction dim
  qT/kT [128o, 4096s] - head dims on partitions (head A: 0-63, head B: 64-127)
  v   [128s, 32kb, 130] - natural layout per key block, with a ones column per
                          head so the PV matmul also produces the softmax
                          denominator (col 64 / col 129).
  scores are computed transposed [k, q]: softmax sum over k comes out of the
  PE via the ones column; masks are multiplicative 0/1 on exp(scores) (safe:
  scores are O(1) here, no max-subtraction needed).
"""

import os
import numpy as np
import ml_dtypes

S = 4096
HIDDEN = 1024
N_CORES = 8
OC = 128          # out-proj contraction dims (head dims) per core = 2 heads x 64
NQB = S // 128    # 32 query/key blocks
BF16 = ml_dtypes.bfloat16

_CACHE = {}
LAST_RESULTS = None


def _masks_np():
    """6 multiplicative masks in scoresT [k(partition), q(free)] layout."""
    p = np.arange(128)[:, None]   # key index within block
    f = np.arange(128)[None, :]   # query index within block
    m_lo = (f <= p)                       # kb == qb-1 window
    m_lo0 = m_lo | (p == 0)               # + global key 0 (qb==1 case)
    m_up = (f >= p)                       # kb == qb+1 window
    m_up0 = m_up | (f == 0)               # + global query 0 (qb==0, kb==1)
    m_row0 = np.broadcast_to(p == 0, (128, 128))   # only global key 0
    m_col0 = np.broadcast_to(f == 0, (128, 128))   # only global query 0
    return np.stack([m_lo, m_lo0, m_up, m_up0, m_row0, m_col0]).astype(BF16)


def _kbs_for(qb):
    """[(key_block, mask_idx or None)] for query block qb."""
    if qb == 0:
        return [(0, None), (1, 3)] + [(kb, 5) for kb in range(2, NQB)]
    if qb == 1:
        return [(0, 1), (1, None), (2, 2)]
    if qb == NQB - 1:
        return [(0, 4), (qb - 1, 0), (qb, None)]
    return [(0, 4), (qb - 1, 0), (qb, None), (qb + 1, 2)]


def _build():
    import concourse.bass as bass
    import concourse.mybir as mybir
    import concourse.tile as tile
    from concourse import bacc

    f32 = mybir.dt.float32
    bf16 = mybir.dt.bfloat16
    Exp = mybir.ActivationFunctionType.Exp

    nc = bacc.Bacc("TRN2", target_bir_lowering=False, debug=False,
                   num_devices=N_CORES)

    xt_d = nc.dram_tensor("xt", [HIDDEN, S], bf16, kind="ExternalInput").ap()
    wq_d = nc.dram_tensor("wqt", [HIDDEN, OC], bf16, kind="ExternalInput").ap()
    wk_d = nc.dram_tensor("wkt", [HIDDEN, OC], bf16, kind="ExternalInput").ap()
    wv_d = nc.dram_tensor("wvt", [HIDDEN, OC], bf16, kind="ExternalInput").ap()
    wo_d = nc.dram_tensor("wot", [OC, HIDDEN], bf16, kind="ExternalInput").ap()
    out_d = nc.dram_tensor("partial", [S, HIDDEN], f32,
                           kind="ExternalOutput").ap()
    mask_d = nc.inline_tensor(_masks_np(), name="masks").ap()
    id_d = nc.inline_tensor(np.eye(128, dtype=BF16), name="ident").ap()

    with tile.TileContext(nc) as tc:
        import contextlib
        with contextlib.ExitStack() as ctx:
            big = ctx.enter_context(tc.tile_pool(name="big", bufs=1))
            tmp = ctx.enter_context(tc.tile_pool(name="tmp", bufs=3))
            psb = ctx.enter_context(tc.tile_pool(name="psb", bufs=3, space="PSUM"))
            pso = ctx.enter_context(tc.tile_pool(name="pso", bufs=2, space="PSUM"))
            pst = ctx.enter_context(tc.tile_pool(name="pst", bufs=2, space="PSUM"))

            # ---- resident tensors ----
            xt_sb = big.tile([128, 8, S], bf16)       # x.T, hidden chunks on dim1
            qt_sb = big.tile([128, S], bf16)          # q.T * 0.125
            kt_sb = big.tile([128, S], bf16)
            v_sb = big.tile([128, NQB, 130], bf16)    # [vA|1|vB|1] per key block
            outn_sb = big.tile([128, NQB, 128], bf16)  # attn out, natural [q, hd]
            outt_sb = big.tile([128, NQB, 128], bf16)  # transposed [hd, q]
            wq_sb = big.tile([128, 8, OC], bf16)
            wk_sb = big.tile([128, 8, OC], bf16)
            wv_sb = big.tile([128, 8, OC], bf16)
            wo_sb = big.tile([128, HIDDEN], bf16)
            mask_sb = big.tile([128, 6, 128], bf16)
            id_sb = big.tile([128, 128], bf16)

            # ---- constant / weight loads ----
            nc.sync.dma_start(wq_sb, wq_d.rearrange("(c p) o -> p c o", p=128))
            nc.sync.dma_start(wk_sb, wk_d.rearrange("(c p) o -> p c o", p=128))
            nc.sync.dma_start(wv_sb, wv_d.rearrange("(c p) o -> p c o", p=128))
            nc.sync.dma_start(wo_sb, wo_d)
            nc.sync.dma_start(mask_sb, mask_d.rearrange("m p f -> p m f"))
            nc.sync.dma_start(id_sb, id_d)
            nc.vector.memset(v_sb[:, :, 64], 1.0)
            nc.vector.memset(v_sb[:, :, 129], 1.0)

            xt_ap = xt_d.rearrange("(c p) s -> p c s", p=128)

            # ---- phase A: projections ----
            for sc in range(8):
                ssl = slice(sc * 512, (sc + 1) * 512)
                nc.sync.dma_start(xt_sb[:, :, ssl], xt_ap[:, :, ssl])

                psq = psb.tile([128, 512], f32, tag="ps512", name="psq")
                for hc in range(8):
                    nc.tensor.matmul(psq, wq_sb[:, hc, :], xt_sb[:, hc, ssl],
                                     start=(hc == 0), stop=(hc == 7))
                # fold the 1/sqrt(hd) = 0.125 softmax scale into q
                nc.vector.tensor_scalar_mul(qt_sb[:, ssl], psq, 0.125)

                psk = psb.tile([128, 512], f32, tag="ps512", name="psk")
                for hc in range(8):
                    nc.tensor.matmul(psk, wk_sb[:, hc, :], xt_sb[:, hc, ssl],
                                     start=(hc == 0), stop=(hc == 7))
                nc.vector.tensor_copy(kt_sb[:, ssl], psk)

                for b in range(4):
                    kb = sc * 4 + b
                    bsl = slice(sc * 512 + b * 128, sc * 512 + b * 128 + 128)
                    psv = psb.tile([128, 512], f32, tag="ps512", name="psv")
                    for hc in range(8):
                        nc.tensor.matmul(psv[:, :128], xt_sb[:, hc, bsl],
                                         wv_sb[:, hc, :],
                                         start=(hc == 0), stop=(hc == 7))
                    nc.vector.tensor_copy(v_sb[:, kb, 0:64], psv[:, 0:64])
                    nc.vector.tensor_copy(v_sb[:, kb, 65:129], psv[:, 64:128])

            # ---- phase B: attention (scoresT layout [k, q]) ----
            for qb in range(NQB):
                qsl = slice(qb * 128, (qb + 1) * 128)
                for h in range(2):
                    bp = 64 * h
                    blocks = _kbs_for(qb)
                    nmm = len(blocks)
                    pso_t = pso.tile([128, 65], f32, tag="psO", name="pso_t")
                    mmi = 0
                    for g0 in range(0, nmm, 4):
                        grp = blocks[g0:g0 + 4]
                        gw = 128 * len(grp)
                        pss = psb.tile([128, 512], f32, tag="ps512", name="pss")
                        for j, (kb, mi) in enumerate(grp):
                            nc.tensor.matmul(
                                pss[:, j * 128:(j + 1) * 128],
                                kt_sb[bp:bp + 64, kb * 128:(kb + 1) * 128],
                                qt_sb[bp:bp + 64, qsl],
                                start=True, stop=True)
                        probs = tmp.tile([128, 512], bf16, tag="probs",
                                         name="probs")
                        nc.scalar.activation(probs[:, :gw], pss[:, :gw], Exp)
                        for j, (kb, mi) in enumerate(grp):
                            if mi is not None:
                                jsl = slice(j * 128, (j + 1) * 128)
                                nc.vector.tensor_mul(probs[:, jsl], probs[:, jsl],
                                                     mask_sb[:, mi, :])
                        for j, (kb, mi) in enumerate(grp):
                            nc.tensor.matmul(
                                pso_t, probs[:, j * 128:(j + 1) * 128],
                                v_sb[:, kb, 65 * h:65 * h + 65],
                                start=(mmi == 0), stop=(mmi == nmm - 1),
                                skip_group_check=True)
                            mmi += 1
                    recip = tmp.tile([128, 1], f32, tag="recip", name="recip")
                    nc.vector.reciprocal(recip, pso_t[:, 64:65])
                    nc.vector.tensor_scalar_mul(
                        outn_sb[:, qb, 64 * h:64 * h + 64],
                        pso_t[:, 0:64], recip)

            # ---- phase C: transpose + out-proj + store ----
            for qb in range(NQB):
                pstr = pst.tile([128, 128], bf16, tag="psT", name="pstr")
                nc.tensor.transpose(pstr, outn_sb[:, qb, :], id_sb)
                nc.vector.tensor_copy(outt_sb[:, qb, :], pstr)
                stage = tmp.tile([128, HIDDEN], f32, tag="stage", name="stage")
                for oc in range(2):
                    psp = psb.tile([128, 512], f32, tag="ps512", name="psp")
                    nc.tensor.matmul(psp, outt_sb[:, qb, :],
                                     wo_sb[:, oc * 512:(oc + 1) * 512],
                                     start=True, stop=True)
                    nc.vector.tensor_copy(stage[:, oc * 512:(oc + 1) * 512], psp)
                nc.sync.dma_start(out_d[qb * 128:(qb + 1) * 128, :], stage)

    nc.compile()
    return nc


def kernel(x, Wq, Wk, Wv, Wo):
    from concourse import bass_utils

    x = np.asarray(x)
    B = x.shape[0]
    xt = np.ascontiguousarray(np.asarray(x)[0].T.astype(BF16))
    in_maps = []
    for d in range(N_CORES):
        rs = slice(OC * d, OC * (d + 1))
        in_maps.append({
            "xt": xt,
            "wqt": np.ascontiguousarray(np.asarray(Wq)[rs, :].T.astype(BF16)),
            "wkt": np.ascontiguousarray(np.asarray(Wk)[rs, :].T.astype(BF16)),
            "wvt": np.ascontiguousarray(np.asarray(Wv)[rs, :].T.astype(BF16)),
            "wot": np.ascontiguousarray(np.asarray(Wo)[:, rs].T.astype(BF16)),
        })

    if "nc" not in _CACHE:
        _CACHE["nc"] = _build()
    nc = _CACHE["nc"]

    res = bass_utils.run_bass_kernel_spmd(
        nc, in_maps, core_ids=list(range(N_CORES)),
        trace=bool(os.environ.get("KERNEL_TRACE")))
    global LAST_RESULTS
    LAST_RESULTS = res

    out = np.zeros((S, HIDDEN), np.float64)
    for r in res.results:
        out += r["partial"].astype(np.float64)
    return out.reshape(B, S, HIDDEN).astype(np.float32)
